# revision 2
# baseline (speedup 1.0000x reference)
"""BuildCostVolume Trainium2 kernel — diagonal-gather formulation.

Reference (per b, n, a):  shear x along d by (32-t) (t=h for uh, w for vw,
zero-fill), then adaptive-avg-pool the centered length-L window
(L = 20*delta+1, delta = max(|a-4|,1)) down to 21 bins:

  out[k,t] = (1/n_k) * sum_{r in [s_k,e_k)} x[(32-10*delta) + r + t, t]

Key insight: only the L diagonal rows G[r,t,:] = x[c+r+t, t, :] of the
sheared tensor are ever touched (c = 32-10*delta).  The host materializes
G per (b,n,a) via numpy as_strided (a pure relayout, like the vw transpose)
so the device reads 7.0MB/core instead of 18.9MB, and the pooling becomes a
single [L->21] x [L,4096] matmul per block with a tiny per-delta weight
matrix.  For delta=1 (a=3,4,5) the pool matrix is the identity, so those 6
blocks/core are pure DMA passthrough (no PE/PSUM).

Device layout per core (b = core index):
  x0/x1  [429, 4096] f16 : G blocks for n=0/1, a-major (row offsets below)
  wsrc   [81, 63]    f16 : P'.T for delta 2,3,4 (21 cols each)
  out_q  [384, 4096] f16 : 3 quads of 4 delta>=2 blocks at partition
                           offsets {0,32,64,96} (rows 21-31 of each 32-row
                           group are garbage, host discards)
  out_p  [126, 4096] f16 : delta=1 passthrough blocks (n0 a3,4,5; n1 a3,4,5)
"""

import numpy as np

import concourse.bass as bass
import concourse.bacc as bacc
import concourse.mybir as mybir
import concourse.tile as tile
from concourse.bass_utils import run_bass_kernel_spmd

F32 = mybir.dt.float32
F16 = mybir.dt.float16

DISP_RANGE = 10
OUT_D = 2 * DISP_RANGE + 1  # 21
B, A, D, H, W = 8, 9, 128, 64, 64
HW = H * W  # 4096
NCORES = 8

DELTA = [max(abs(a - A // 2), 1) for a in range(A)]  # [4,3,2,1,1,1,2,3,4]
LS = [2 * DISP_RANGE * d + 1 for d in DELTA]  # [81,61,41,21,21,21,41,61,81]
ROWOFF = np.concatenate([[0], np.cumsum(LS)]).tolist()  # offsets into x0/x1
XROWS = ROWOFF[-1]  # 429
WSLOT = {2: 0, 3: 1, 4: 2}  # delta -> 21-col slot in wsrc

# delta>=2 blocks packed 4 per quad at PSUM partition offsets 0/32/64/96
QUADS = [
    [(0, 0), (0, 1), (0, 2), (0, 6)],
    [(0, 7), (0, 8), (1, 0), (1, 1)],
    [(1, 2), (1, 6), (1, 7), (1, 8)],
]
PASS_A = [3, 4, 5]  # delta=1 blocks, contiguous rows 183:246 in x0/x1

TRACE = False  # set by test.py for profiling runs
LAST_RESULTS = None  # BassKernelResults of the most recent run

_COMPILED = None


def _pool_matrix():
    # [9, 21, 128]; same as reference._pool_matrix(9, 128)
    P = np.zeros((A, OUT_D, D), dtype=np.float32)
    for i in range(A):
        a_delta = max(abs(i - A // 2), 1)
        L = 2 * DISP_RANGE * a_delta + 1
        start0 = D // 2 - DISP_RANGE * a_delta
        for k in range(OUT_D):
            s = (k * L) // OUT_D
            e = -((-(k + 1) * L) // OUT_D)
            P[i, k, start0 + s : start0 + e] = 1.0 / (e - s)
    return P


def _build_wsrc():
    # [81, 63]: P'.T for delta 2,3,4; P'[k, r] = P[a][k, 64-10*delta+r]
    P = _pool_matrix()
    wsrc = np.zeros((81, 63), dtype=np.float32)
    for delta, a_repr in [(2, 2), (3, 1), (4, 0)]:
        L = 2 * DISP_RANGE * delta + 1
        s0 = 64 - DISP_RANGE * delta
        wsrc[0:L, 21 * WSLOT[delta] : 21 * WSLOT[delta] + 21] = P[a_repr][
            :, s0 : s0 + L
        ].T
    return wsrc.astype(np.float16)


def _build_nc():
    nc = bacc.Bacc("TRN2", target_bir_lowering=False)

    x0 = nc.declare_dram_parameter("x0", [XROWS, HW], F16, isOutput=False)
    x1 = nc.declare_dram_parameter("x1", [XROWS, HW], F16, isOutput=False)
    wsrc = nc.declare_dram_parameter("wsrc", [81, 63], F16, isOutput=False)
    out_q = nc.declare_dram_parameter("out_q", [3 * 128, HW], F16, isOutput=True)
    out_p = nc.declare_dram_parameter("out_p", [2 * 63, HW], F16, isOutput=True)

    xs = [x0, x1]

    with tile.TileContext(nc) as tc:
        with (
            tc.tile_pool(name="wpool", bufs=1) as wp,
            tc.tile_pool(name="xpool", bufs=12) as xp,
            tc.tile_pool(name="ppool", bufs=2) as ppass,
            tc.tile_pool(name="opool", bufs=3) as op,
            tc.tile_pool(name="psum", bufs=8, space="PSUM") as pp,
        ):
            wt = wp.tile([81, 63], F16, tag="w", name="wt")
            nc.scalar.dma_start(out=wt[:], in_=wsrc[:])

            # Loads: quad 0 first (compute starts earliest), then the
            # passthrough blocks (their stores overlap quad compute),
            # then quads 1-2.
            gts = {}
            for n, a in QUADS[0]:
                gt = xp.tile([LS[a], HW], F16, tag="g", name=f"g{n}_{a}")
                nc.sync.dma_start(out=gt[:], in_=xs[n][ROWOFF[a] : ROWOFF[a] + LS[a]])
                gts[(n, a)] = gt
            pts_pass = []
            for n in (0, 1):
                t = ppass.tile([63, HW], F16, tag="p", name=f"pass{n}")
                nc.sync.dma_start(
                    out=t[:], in_=xs[n][ROWOFF[3] : ROWOFF[3] + 63]
                )
                pts_pass.append(t)
            for quad in QUADS[1:]:
                for n, a in quad:
                    gt = xp.tile([LS[a], HW], F16, tag="g", name=f"g{n}_{a}")
                    nc.sync.dma_start(
                        out=gt[:], in_=xs[n][ROWOFF[a] : ROWOFF[a] + LS[a]]
                    )
                    gts[(n, a)] = gt

            # Passthrough stores (pure DMA, start while quads compute).
            for n in (0, 1):
                nc.gpsimd.dma_start(
                    out=out_p[63 * n : 63 * n + 63], in_=pts_pass[n][:]
                )

            for qi, quad in enumerate(QUADS):
                psts = [
                    pp.tile([128, 512], F32, tag="ps", name=f"ps{qi}_{c}")
                    for c in range(8)
                ]
                # Bank-major emission: all 4 writes to bank c happen
                # consecutively so its copy can start early.
                for c in range(8):
                    for ti, (n, a) in enumerate(quad):
                        L = LS[a]
                        ws = 21 * WSLOT[DELTA[a]]
                        p0 = 32 * ti
                        nc.tensor.matmul(
                            out=psts[c][p0 : p0 + OUT_D, :],
                            lhsT=wt[0:L, ws : ws + 21],
                            rhs=gts[(n, a)][:, 512 * c : 512 * c + 512],
                            start=True,
                            stop=True,
                            tile_position=(0, p0),
                        )
                osb = op.tile([128, HW], F16, tag="o", name=f"osb{qi}")
                for c in range(8):
                    dst = osb[:, 512 * c : 512 * c + 512]
                    if c % 2 == 0:
                        nc.vector.tensor_copy(out=dst, in_=psts[c][:])
                    else:
                        nc.scalar.copy(out=dst, in_=psts[c][:])
                nc.gpsimd.dma_start(
                    out=out_q[128 * qi : 128 * qi + 128], in_=osb[:]
                )

    nc.compile()
    return nc


def _get_compiled():
    global _COMPILED
    if _COMPILED is None:
        _COMPILED = _build_nc()
    return _COMPILED


def _gather_blocks(xp_pad):
    """xp_pad: [B, A, 144, 64, 64] f16 (zero-padded d axis).
    Returns [B, XROWS, 4096] f16: per a, G[r,t,u] = x[c+r+t, t, u]."""
    out = np.empty((B, XROWS, HW), dtype=np.float16)
    sb, sa, s0, s1, s2 = xp_pad.strides
    for a in range(A):
        L = LS[a]
        c = 32 - 10 * DELTA[a]
        base = xp_pad[:, a, c + 8 :]
        G = np.lib.stride_tricks.as_strided(
            base, shape=(B, L, 64, 64), strides=(sb, s0, s0 + s1, s2)
        )
        out[:, ROWOFF[a] : ROWOFF[a] + L] = G.reshape(B, L, HW)
    return out


def kernel(attn_map_uh, attn_map_vw):
    global LAST_RESULTS
    uh16 = np.asarray(attn_map_uh, dtype=np.float16)
    vwt16 = np.swapaxes(np.asarray(attn_map_vw), -1, -2).astype(np.float16)

    pad = np.zeros((2, B, A, 144, H, W), dtype=np.float16)
    pad[0, :, :, 8 : 8 + D] = uh16
    pad[1, :, :, 8 : 8 + D] = vwt16
    xg0 = _gather_blocks(pad[0])
    xg1 = _gather_blocks(pad[1])
    wsrc = _build_wsrc()

    nc = _get_compiled()
    in_maps = [
        {"x0": xg0[c], "x1": xg1[c], "wsrc": wsrc} for c in range(NCORES)
    ]
    res = run_bass_kernel_spmd(nc, in_maps, list(range(NCORES)), trace=TRACE)
    LAST_RESULTS = res

    out16 = np.empty((B, 2, A, OUT_D, H, W), dtype=np.float16)
    for c in range(NCORES):
        oq = res.results[c]["out_q"].reshape(3, 4, 32, H, W)
        opp = res.results[c]["out_p"].reshape(2, 3, OUT_D, H, W)
        for qi, quad in enumerate(QUADS):
            for ti, (n, a) in enumerate(quad):
                blk = oq[qi, ti, :OUT_D]
                out16[c, n, a] = blk if n == 0 else np.swapaxes(blk, -1, -2)
        for n in (0, 1):
            for ai, a in enumerate(PASS_A):
                blk = opp[n, ai]
                out16[c, n, a] = blk if n == 0 else np.swapaxes(blk, -1, -2)
    return out16.astype(np.float32)


# revision 4
# speedup vs baseline: 3.4654x; 3.4654x over previous
"""BuildCostVolume Trainium2 kernel — diagonal-gather formulation.

Reference (per b, n, a):  shear x along d by (32-t) (t=h for uh, w for vw,
zero-fill), then adaptive-avg-pool the centered length-L window
(L = 20*delta+1, delta = max(|a-4|,1)) down to 21 bins:

  out[k,t] = (1/n_k) * sum_{r in [s_k,e_k)} x[(32-10*delta) + r + t, t]

Key insight: only the L diagonal rows G[r,t,:] = x[c+r+t, t, :] of the
sheared tensor are ever touched (c = 32-10*delta).  The host materializes
G per (b,n,a) via numpy as_strided (a pure relayout, like the vw transpose)
so the device reads ~9.4MB/core instead of 18.9MB, and the pooling becomes
a single [L->21] x [L,4096] matmul per block with a tiny per-delta weight
matrix.  For delta=1 (a=3,4,5) the pool matrix is the identity, so those 6
blocks/core are pure DMA passthrough (no PE/PSUM).

DMA engine-split is only even for 128-partition transfers (odd partition
counts lump most packets onto one engine), so the 18 variable-K blocks are
bin-packed into nine [128,4096] SBUF tiles at 32-aligned partition bases
(matmul tile_position row must be 0 / {0,64} / {0,32,64,96} for K>64 /
K<=64 / K<=32).  Weights are replicated per (delta, base) combo.

Device layout per core (b = core index):
  xg    [1152, 4096] f16 : 9 packed tiles of gathered G blocks
  wsrc  [128, 105]   f16 : P'.T at each (delta, base) combo used
  out_q [384, 4096]  f16 : 3 quads of 4 delta>=2 blocks at partition
                           offsets {0,32,64,96} (rows 21-31 of each 32-row
                           group are garbage, host discards)
  out_p [126, 4096]  f16 : delta=1 passthrough blocks
"""

import numpy as np

import concourse.bass as bass
import concourse.bacc as bacc
import concourse.mybir as mybir
import concourse.tile as tile
from concourse.bass_utils import run_bass_kernel_spmd

F32 = mybir.dt.float32
F16 = mybir.dt.float16

DISP_RANGE = 10
OUT_D = 2 * DISP_RANGE + 1  # 21
B, A, D, H, W = 8, 9, 128, 64, 64
HW = H * W  # 4096
NCORES = 8

DELTA = [max(abs(a - A // 2), 1) for a in range(A)]  # [4,3,2,1,1,1,2,3,4]
LS = [2 * DISP_RANGE * d + 1 for d in DELTA]  # [81,61,41,21,21,21,41,61,81]

# Bin-packing of the 18 (n, a) blocks into nine 128-row tiles.
# Each entry: (n, a, base_partition).
TILES = [
    [(0, 0, 0), (0, 3, 96)],
    [(0, 8, 0), (0, 4, 96)],
    [(1, 0, 0), (0, 5, 96)],
    [(1, 8, 0), (1, 3, 96)],
    [(1, 4, 0), (1, 5, 32)],
    [(0, 1, 0), (0, 7, 64)],
    [(1, 1, 0), (1, 7, 64)],
    [(0, 2, 0), (0, 6, 64)],
    [(1, 2, 0), (1, 6, 64)],
]
NTILES = len(TILES)
XROWS = 128 * NTILES  # 1152

# (tile_idx, base) per block
BLOCK_SLOT = {(n, a): (t, base) for t, tl in enumerate(TILES) for n, a, base in tl}

# Weight combos (delta, base) used by the delta>=2 matmuls.
WCOMBOS = [(4, 0), (3, 0), (3, 64), (2, 0), (2, 64)]
WCOL = {c: 21 * i for i, c in enumerate(WCOMBOS)}
WCOLS = 21 * len(WCOMBOS)  # 105

# delta>=2 blocks packed 4 per quad at PSUM partition offsets 0/32/64/96
QUADS = [
    [(0, 0), (0, 8), (1, 0), (1, 8)],
    [(0, 1), (0, 7), (1, 1), (1, 7)],
    [(0, 2), (0, 6), (1, 2), (1, 6)],
]
# delta=1 passthrough blocks; out_p rows 21*i : 21*i+21
PASS = [(0, 3), (0, 4), (0, 5), (1, 3), (1, 4), (1, 5)]

TRACE = False  # set by test.py for profiling runs
LAST_RESULTS = None  # BassKernelResults of the most recent run

_COMPILED = None


def _pool_matrix():
    # [9, 21, 128]; same as reference._pool_matrix(9, 128)
    P = np.zeros((A, OUT_D, D), dtype=np.float32)
    for i in range(A):
        a_delta = max(abs(i - A // 2), 1)
        L = 2 * DISP_RANGE * a_delta + 1
        start0 = D // 2 - DISP_RANGE * a_delta
        for k in range(OUT_D):
            s = (k * L) // OUT_D
            e = -((-(k + 1) * L) // OUT_D)
            P[i, k, start0 + s : start0 + e] = 1.0 / (e - s)
    return P


def _build_wsrc():
    # [128, 105]: per combo (delta, base), rows base..base+L hold P'.T
    # with P'[k, r] = P[a][k, 64-10*delta+r].
    P = _pool_matrix()
    arepr = {2: 2, 3: 1, 4: 0}
    wsrc = np.zeros((128, WCOLS), dtype=np.float32)
    for delta, base in WCOMBOS:
        L = 2 * DISP_RANGE * delta + 1
        s0 = 64 - DISP_RANGE * delta
        col = WCOL[(delta, base)]
        wsrc[base : base + L, col : col + 21] = P[arepr[delta]][:, s0 : s0 + L].T
    return wsrc.astype(np.float16)


def _build_nc():
    nc = bacc.Bacc("TRN2", target_bir_lowering=False)

    xg = nc.declare_dram_parameter("xg", [XROWS, HW], F16, isOutput=False)
    wsrc = nc.declare_dram_parameter("wsrc", [128, WCOLS], F16, isOutput=False)
    out_q = nc.declare_dram_parameter("out_q", [3 * 128, HW], F16, isOutput=True)
    out_p = nc.declare_dram_parameter("out_p", [126, HW], F16, isOutput=True)

    with tile.TileContext(nc) as tc:
        with (
            tc.tile_pool(name="wpool", bufs=1) as wp,
            tc.tile_pool(name="xpool", bufs=NTILES) as xp,
            tc.tile_pool(name="opool", bufs=3) as op,
            tc.tile_pool(name="psum", bufs=8, space="PSUM") as pp,
        ):
            wt = wp.tile([128, WCOLS], F16, tag="w", name="wt")
            nc.scalar.dma_start(out=wt[:], in_=wsrc[:])

            xts = []
            for t in range(NTILES):
                xt = xp.tile([128, HW], F16, tag="g", name=f"xt{t}")
                nc.sync.dma_start(out=xt[:], in_=xg[128 * t : 128 * t + 128])
                xts.append(xt)

            # Passthrough stores (pure DMA, overlap quad compute).
            for i, (n, a) in enumerate(PASS):
                t, base = BLOCK_SLOT[(n, a)]
                nc.gpsimd.dma_start(
                    out=out_p[21 * i : 21 * i + 21],
                    in_=xts[t][base : base + OUT_D, :],
                )

            for qi, quad in enumerate(QUADS):
                psts = [
                    pp.tile([128, 512], F32, tag="ps", name=f"ps{qi}_{c}")
                    for c in range(8)
                ]
                # Bank-major emission: all 4 writes to bank c happen
                # consecutively so its copy can start early.
                for c in range(8):
                    for ti, (n, a) in enumerate(quad):
                        L = LS[a]
                        t, base = BLOCK_SLOT[(n, a)]
                        wc = WCOL[(DELTA[a], base)]
                        p0 = 32 * ti
                        nc.tensor.matmul(
                            out=psts[c][p0 : p0 + OUT_D, :],
                            lhsT=wt[base : base + L, wc : wc + 21],
                            rhs=xts[t][base : base + L, 512 * c : 512 * c + 512],
                            start=True,
                            stop=True,
                            tile_position=(base, p0),
                        )
                osb = op.tile([128, HW], F16, tag="o", name=f"osb{qi}")
                for c in range(8):
                    dst = osb[:, 512 * c : 512 * c + 512]
                    if c % 2 == 0:
                        nc.vector.tensor_copy(out=dst, in_=psts[c][:])
                    else:
                        nc.scalar.copy(out=dst, in_=psts[c][:])
                nc.gpsimd.dma_start(
                    out=out_q[128 * qi : 128 * qi + 128], in_=osb[:]
                )

    nc.compile()
    return nc


def _get_compiled():
    global _COMPILED
    if _COMPILED is None:
        _COMPILED = _build_nc()
    return _COMPILED


def _gather_packed(pad):
    """pad: [2, B, A, 144, 64, 64] f16 (zero-padded d axis, n=1 transposed).
    Returns [B, XROWS, 4096] f16 per the TILES packing:
    G[r,t,u] = x[c+r+t, t, u], c = 32-10*delta."""
    out = np.zeros((B, XROWS, HW), dtype=np.float16)
    _, sb, _, s0, s1, s2 = pad.strides
    for t, tl in enumerate(TILES):
        for n, a, base in tl:
            L = LS[a]
            c = 32 - 10 * DELTA[a]
            src = pad[n, :, a, c + 8 :]
            G = np.lib.stride_tricks.as_strided(
                src, shape=(B, L, 64, 64), strides=(sb, s0, s0 + s1, s2)
            )
            out[:, 128 * t + base : 128 * t + base + L] = G.reshape(B, L, HW)
    return out


def kernel(attn_map_uh, attn_map_vw):
    global LAST_RESULTS
    uh16 = np.asarray(attn_map_uh, dtype=np.float16)
    vwt16 = np.swapaxes(np.asarray(attn_map_vw), -1, -2).astype(np.float16)

    pad = np.zeros((2, B, A, 144, H, W), dtype=np.float16)
    pad[0, :, :, 8 : 8 + D] = uh16
    pad[1, :, :, 8 : 8 + D] = vwt16
    xg = _gather_packed(pad)
    wsrc = _build_wsrc()

    nc = _get_compiled()
    in_maps = [{"xg": xg[c], "wsrc": wsrc} for c in range(NCORES)]
    res = run_bass_kernel_spmd(nc, in_maps, list(range(NCORES)), trace=TRACE)
    LAST_RESULTS = res

    out16 = np.empty((B, 2, A, OUT_D, H, W), dtype=np.float16)
    for c in range(NCORES):
        oq = res.results[c]["out_q"].reshape(3, 4, 32, H, W)
        opp = res.results[c]["out_p"].reshape(6, OUT_D, H, W)
        for qi, quad in enumerate(QUADS):
            for ti, (n, a) in enumerate(quad):
                blk = oq[qi, ti, :OUT_D]
                out16[c, n, a] = blk if n == 0 else np.swapaxes(blk, -1, -2)
        for i, (n, a) in enumerate(PASS):
            blk = opp[i]
            out16[c, n, a] = blk if n == 0 else np.swapaxes(blk, -1, -2)
    return out16.astype(np.float32)
